# revision 1
# baseline (speedup 1.0000x reference)
"""Trainium2 Bass kernel v2 for nn_DenseFlashAttention_16123307229343
(GNN segment-softmax message passing). kernel(**inputs) -> np.ndarray.

v2 redesign vs baseline:
- w_out folded into projection weights on host (Wcat' = 0.25*W_h@w_out),
  killing the per-chunk transpose + output matmul epilogue.
- One-hot scatter matrices built on host, DMA'd (no is_equal on DVE).
- edge_len carried as a 65th contraction row so the radial bias comes out
  of the projection matmul for free.
- Weighted payload built with 16-wide-sliced bf16 tensor_tensor ops that
  keep step-1 access (2x_1P DVE mode) instead of 1x broadcast ops.
- Denominators ride as column 65 of each head block: scatter stays
  2 matmuls/tile of N=260.
- All matmuls bf16 (FWL weight loads).
"""
import sys, math
sys.path.insert(0, '/opt/trn_rl_repo')

import numpy as np
import ml_dtypes

import concourse.mybir as mybir
import concourse.bass as bass
from concourse.tile import TileContext
from concourse.vector_clock import ScopedClock

bf16 = ml_dtypes.bfloat16
FP32 = mybir.dt.float32
BF16 = mybir.dt.bfloat16
ALU = mybir.AluOpType
ACTF = mybir.ActivationFunctionType

MAXW = 1


def _patched_drain_and_barrier(self, tick_clock, wait_clock):
    nc = self.nc
    drain_inst = nc.sync.drain()
    wait_clock.add_sem_waits(drain_inst.ins, ScopedClock({None: tick_clock.global_clock}))
    si = drain_inst.ins.sync_info
    waits = list(si.on_wait) if si is not None else []
    if len(waits) > MAXW:
        si.on_wait = waits[:MAXW]
        rest = waits[MAXW:]
        for i in range(0, len(rest), MAXW):
            d2 = nc.sync.drain()
            d2.ins.sync_info = mybir.SyncInfo(on_wait=rest[i:i+MAXW], on_update=[])
    nc.all_engine_barrier()
    popped = nc._tile_sem_poison_stack.pop()
    assert popped is self._sem_poison
    nc.clear_and_free_semaphores(list(self.sems.allocated().values()))
    nc.all_engine_barrier()


def install():
    TileContext._drain_and_barrier = _patched_drain_and_barrier


_ctr = [0]


def split_sync_waits(nc, maxw=1):
    """The walrus build in this container rejects instructions carrying more
    than one sync wait. Hoist extra waits onto carriers inserted immediately
    before the instruction on the same engine."""
    for f in nc.m.functions:
        for blk in f.blocks:
            lst = blk.instructions
            i = 0
            while i < len(lst):
                ins = lst[i]
                si = ins.sync_info
                if si is None:
                    i += 1
                    continue
                waits = list(si.on_wait)
                if len(waits) <= maxw:
                    i += 1
                    continue
                si.on_wait = waits[-maxw:]
                rest = waits[:-maxw]
                carriers = []
                for j in range(0, len(rest), maxw):
                    _ctr[0] += 1
                    nop = mybir.InstEventSemaphore(name=f"waitnop_{_ctr[0]}", ins=[], outs=[])
                    nop.engine = ins.engine
                    nop.sync_info = mybir.SyncInfo(on_wait=rest[j:j + maxw],
                                                   on_update=[])
                    nc.register_instruction(nop, overwrite=True)
                    carriers.append(nop)
                for k, nop in enumerate(carriers):
                    lst.insert(i + k, nop)
                i += len(carriers) + 1


class Params:
    def __init__(self, N, E, F=64, H=4, NC=8, GC=2):
        self.N, self.E, self.F, self.H, self.NC = N, E, F, H, NC
        self.RPC = N // NC          # receivers per core
        ch = math.ceil(self.RPC / 128)
        self.GC = GC                # chunks per group
        self.CH = math.ceil(ch / GC) * GC
        self.SLOTS = self.CH * 128
        self.NG = self.CH // GC


def host_prep(p: Params, x, edge_index, edge_len):
    N, NC = p.N, p.NC
    snd = edge_index[0].astype(np.int64)
    rcv = edge_index[1].astype(np.int64)
    deg = np.bincount(rcv, minlength=N)
    order = np.argsort(rcv, kind='stable')
    starts = np.zeros(N + 1, np.int64)
    np.cumsum(deg, out=starts[1:])

    cores = []
    T_need = 0
    for k in range(NC):
        lo, hi = k * p.RPC, (k + 1) * p.RPC
        d = deg[lo:hi]
        srt = np.argsort(-d, kind='stable')
        bin_of = np.empty(p.RPC, np.int64)
        slot_of = np.empty(p.RPC, np.int64)
        ar = np.arange(p.RPC)
        bin_of[srt] = ar % p.CH
        slot_of[srt] = ar // p.CH
        bin_edge_counts = np.zeros(p.CH, np.int64)
        np.add.at(bin_edge_counts, bin_of, d)
        T_need = max(T_need, int(np.ceil(bin_edge_counts.max() / 128)))
        cores.append(dict(bin_of=bin_of, slot_of=slot_of))
    T = max(T_need, 1)

    per_core = []
    for k in range(NC):
        c = cores[k]
        lo = k * p.RPC
        TT = p.CH * T
        esnd = np.full((TT * 128,), -1, np.int64)       # -1 = dummy
        lens = np.zeros((TT * 128,), np.float32)
        roff = np.full((TT * 128,), 255, np.int64)      # 255 = dummy slot
        fill = np.zeros(p.CH, np.int64)
        for r_local in np.argsort(c['bin_of'], kind='stable'):
            b = c['bin_of'][r_local]
            s = c['slot_of'][r_local]
            n = lo + r_local
            e0, e1 = starts[n], starts[n + 1]
            cnt = e1 - e0
            if cnt == 0:
                continue
            base = b * (T * 128) + fill[b]
            eidx = order[e0:e1]
            esnd[base:base + cnt] = snd[eidx]
            lens[base:base + cnt] = edge_len[eidx]
            roff[base:base + cnt] = s
            fill[b] += cnt
        assert fill.max() <= T * 128
        # xe_aug [65, TT*128]: rows 0:64 = x[snd[e]].T, row 64 = edge_len
        xe = np.zeros((TT * 128, p.F), np.float32)
        real = esnd >= 0
        xe[real] = x[esnd[real]]
        xeT = np.zeros((p.F + 1, TT * 128), np.float32)
        xeT[0:p.F] = xe.T
        xeT[p.F] = lens
        # one-hot scatter lhsT, per tile: ohT[e, t*128 + s]
        ohT = np.zeros((128, TT * 128), np.float32)
        pos = np.arange(TT * 128)
        rmask = roff < 255
        ohT[pos[rmask] % 128, (pos[rmask] // 128) * 128 + roff[rmask]] = 1.0
        x_rcv = np.zeros((p.SLOTS, p.F), np.float32)
        slot_global = c['bin_of'] * 128 + c['slot_of']
        x_rcv[slot_global] = x[lo:lo + p.RPC]
        x_rcvT = np.ascontiguousarray(x_rcv.T)
        per_core.append(dict(xeT=xeT.astype(bf16), ohT=ohT.astype(bf16),
                             x_rcv=x_rcv, x_rcvT=x_rcvT.astype(bf16),
                             slot_global=slot_global))
    return dict(T=T, per_core=per_core)


def build_program(p: Params, T: int):
    nc = bass.Bass("TRN2", target_bir_lowering=False, debug=False,
                   num_devices=p.NC)
    F, H, CH, GC, NG = p.F, p.H, p.CH, p.GC, p.NG
    S = p.SLOTS
    TT = CH * T
    T2 = GC * T                      # tiles per group (even)
    BW = F + 2                       # 66: block width (even, keeps 4B align)
    RW = 4 * BW                      # 264: 4 head-blocks of [64 pay|1 den|1 pad]

    xeT = nc.dram_tensor("xeT", [F + 1, TT * 128], BF16, kind="ExternalInput").ap()
    ohT = nc.dram_tensor("ohT", [128, TT * 128], BF16, kind="ExternalInput").ap()
    wv = nc.dram_tensor("wv", [F + 1, 4 * F + 2 * H], BF16, kind="ExternalInput").ap()
    wm2 = nc.dram_tensor("wm2", [F, F], BF16, kind="ExternalInput").ap()
    x_rcvT = nc.dram_tensor("x_rcvT", [F, S], BF16, kind="ExternalInput").ap()
    x_rcv = nc.dram_tensor("x_rcv", [S, F], FP32, kind="ExternalInput").ap()
    y_perm = nc.dram_tensor("y_perm", [S, F], FP32, kind="ExternalOutput").ap()

    with TileContext(nc) as tc:
        import contextlib
        ctx = contextlib.ExitStack()
        with ctx:
            const = ctx.enter_context(tc.tile_pool(name="const", bufs=1))
            wv_s = const.tile([F + 1, 4 * F + 2 * H], BF16)
            nc.sync.dma_start(out=wv_s[:], in_=wv[:])
            wm2_s = const.tile([F, F], BF16)
            nc.sync.dma_start(out=wm2_s[:], in_=wm2[:])
            xrT_s = const.tile([F, S], BF16)
            nc.sync.dma_start(out=xrT_s[:], in_=x_rcvT[:])

            with tc.tile_pool(name="ep", bufs=2) as ep, \
                 tc.tile_pool(name="big", bufs=2) as bigp, \
                 tc.tile_pool(name="rp", bufs=2) as rp, \
                 tc.tile_pool(name="pep", bufs=2, space="PSUM") as pep, \
                 tc.tile_pool(name="cps", bufs=2, space="PSUM") as cps:

                state = {}

                def stage_p1(g):
                    """Generator: DMA + per-ST proj/scores MMs + ACT staging.
                    Yields after each ST so scatter MMs can interleave."""
                    t0 = g * T2
                    xe_g = ep.tile([F + 1, T2 * 128], BF16, tag="xe")
                    nc.sync.dma_start(out=xe_g[:],
                                      in_=xeT[:, t0 * 128:(t0 + T2) * 128])
                    oh_g = ep.tile([128, T2 * 128], BF16, tag="oh")
                    nc.sync.dma_start(out=oh_g[:],
                                      in_=ohT[:, t0 * 128:(t0 + T2) * 128])
                    projb = bigp.tile([128, T2, H, F], BF16, tag="projb")
                    wts = bigp.tile([128, T2, 2 * H], BF16, tag="wts")
                    state[g] = (oh_g, projb, wts)
                    for st in range(T2 // 2):
                        # [128, 2 banks, 8, 64]: blocks 0:4 proj, block 4
                        # cols 0:8 scores (bank stride 512 = bank-aligned)
                        pe_st = pep.tile([128, 2, 8, F], FP32,
                                         space="PSUM", tag="pe")
                        for i in range(2):
                            t = st * 2 + i
                            nc.tensor.matmul(
                                out=pe_st[:, i].rearrange(
                                    "p a b -> p (a b)")[:, 0:4 * F + 2 * H],
                                lhsT=xe_g[:, t * 128:(t + 1) * 128],
                                rhs=wv_s[:], start=True, stop=True)
                        # wts = exp(scores) first ([tang(4) | rad(4)];
                        # radial len-bias already added via row 64)
                        nc.scalar.activation(
                            out=wts[:, st * 2:st * 2 + 2, :],
                            in_=pe_st[:, :, H, 0:2 * H],
                            func=ACTF.Exp)
                        # proj' -> bf16 staging (ScalarE, PSUM-adjacent)
                        nc.scalar.copy(
                            out=projb[:, st * 2:st * 2 + 2, :, :],
                            in_=pe_st[:, :, 0:H, :])
                        yield

                def stage_p2_half(g, hf):
                    """Weight replication + weighted-payload multiplies for
                    tile range [hf*T, (hf+1)*T) (one chunk's tiles)."""
                    oh_g, projb, wts, wrepA, wrepB, pay = state[g]
                    lo, hi = hf * T, (hf + 1) * T
                    nc.scalar.copy(
                        out=wrepA[:, lo:hi],
                        in_=wts[:, lo:hi, 0:H, None].to_broadcast(
                            [128, T, H, 16]))
                    nc.vector.tensor_copy(
                        out=wrepB[:, lo:hi],
                        in_=wts[:, lo:hi, H:2 * H, None].to_broadcast(
                            [128, T, H, 16]))
                    for ty in range(2):
                        wrep = wrepA if ty == 0 else wrepB
                        for fo in range(F // 16):
                            nc.vector.tensor_tensor(
                                out=pay[:, lo:hi, ty * H:(ty + 1) * H,
                                        fo * 16:(fo + 1) * 16],
                                in0=projb[:, lo:hi, :, fo * 16:(fo + 1) * 16],
                                in1=wrep[:, lo:hi], op=ALU.mult)
                    nc.vector.tensor_copy(out=pay[:, lo:hi, :, F],
                                          in_=wts[:, lo:hi])

                def stage_p2(g):
                    oh_g, projb, wts = state[g]
                    wrepA = bigp.tile([128, T2, H, 16], BF16, tag="wrepA")
                    wrepB = bigp.tile([128, T2, H, 16], BF16, tag="wrepB")
                    # pay [128, T2, 2H, BW]: blocks j = (type, h);
                    # col F of each block holds w_j (denominator column)
                    pay = bigp.tile([128, T2, 2 * H, BW], BF16, tag="pay")
                    state[g] = (oh_g, projb, wts, wrepA, wrepB, pay)
                    for hf in range(GC):
                        stage_p2_half(g, hf)
                    state[g] = (oh_g, pay)

                def scatter_tile(g, i, ps_pair):
                    """Two scatter MMs for global scatter-step i of group g."""
                    oh_g, pay = state[g]
                    cc, k = divmod(i, T)
                    ps = ps_pair[cc]
                    t = cc * T + k
                    nc.tensor.matmul(
                        out=ps[:, 0, 0:RW],
                        lhsT=oh_g[:, t * 128:(t + 1) * 128],
                        rhs=pay[:, t, 0:H, :],
                        start=(k == 0), stop=(k == T - 1))
                    nc.tensor.matmul(
                        out=ps[:, 1, 0:RW],
                        lhsT=oh_g[:, t * 128:(t + 1) * 128],
                        rhs=pay[:, t, H:2 * H, :],
                        start=(k == 0), stop=(k == T - 1))

                def epilogue_chunk(g, cc, ps):
                    if True:
                        ch = g * GC + cc
                        # wm2 term into spare cols of bank 1
                        nc.tensor.matmul(
                            out=ps[:, 1, RW:RW + F],
                            lhsT=xrT_s[:, ch * 128:(ch + 1) * 128],
                            rhs=wm2_s[:], start=True, stop=True,
                            skip_group_check=True)
                        den = rp.tile([128, 2, H], FP32, tag="den")
                        nc.vector.tensor_scalar(
                            out=den[:],
                            in0=ps[:, :, F:H * BW:BW],
                            scalar1=1e-30, scalar2=None, op0=ALU.max)
                        rcp = rp.tile([128, 2, H], FP32, tag="rcp")
                        nc.vector.reciprocal(out=rcp[:], in_=den[:])
                        mneg = rp.tile([128, 1], FP32, tag="mneg")
                        nc.vector.tensor_scalar(
                            out=mneg[:], in0=den[:, 0, 0:1],
                            scalar1=2e-30, scalar2=None, op0=ALU.is_gt)
                        # tmp[j] = U'_j * rcp_j (one strided op, both banks)
                        tmp = rp.tile([128, 2, H, F], BF16, tag="tmp")
                        nc.vector.tensor_tensor(
                            out=tmp[:],
                            in0=ps[:, :, 0:H * BW].rearrange(
                                "p b (h f1) -> p b h f1", h=H)[:, :, :, 0:F],
                            in1=rcp[:, :, :, None].to_broadcast(
                                [128, 2, H, F]),
                            op=ALU.mult)
                        # sum over 8 blocks (tree)
                        r1 = rp.tile([128, H, F], BF16, tag="r1")
                        nc.vector.tensor_tensor(out=r1[:], in0=tmp[:, 0],
                                                in1=tmp[:, 1], op=ALU.add)
                        r2 = rp.tile([128, 2, F], BF16, tag="r2")
                        nc.vector.tensor_tensor(out=r2[:], in0=r1[:, 0:2, :],
                                                in1=r1[:, 2:4, :], op=ALU.add)
                        r3 = rp.tile([128, F], BF16, tag="r3")
                        nc.vector.tensor_tensor(out=r3[:], in0=r2[:, 0],
                                                in1=r2[:, 1], op=ALU.add)
                        # receiver-side term, gated for empty receivers
                        accp = rp.tile([128, F], BF16, tag="accp")
                        nc.scalar.mul(accp[:], ps[:, 1, RW:RW + F],
                                      mneg[:, 0:1])
                        yb = rp.tile([128, F], FP32, tag="yb")
                        nc.vector.tensor_tensor(out=yb[:], in0=r3[:],
                                                in1=accp[:], op=ALU.add)
                        xr = rp.tile([128, F], FP32, tag="xr")
                        nc.sync.dma_start(
                            out=xr[:], in_=x_rcv[ch * 128:(ch + 1) * 128, :])
                        ybf = rp.tile([128, F], FP32, tag="ybf")
                        nc.vector.tensor_tensor(out=ybf[:], in0=yb[:],
                                                in1=xr[:], op=ALU.add)
                        nc.sync.dma_start(
                            out=y_perm[ch * 128:(ch + 1) * 128, :], in_=ybf[:])

                # software pipeline: group g+1 staging interleaves with
                # group g scatter so the PE stream never has a >3us gap
                # (HAM stays warm) and ACT copies hide under scatter MMs.
                def stage(g):
                    for _ in stage_p1(g):
                        pass
                    stage_p2(g)

                stage(0)
                for g in range(NG):
                    if g + 1 < NG:
                        stage(g + 1)
                    ps_pair = []
                    for i in range(GC * T):
                        if i % T == 0:
                            ps = cps.tile([128, 2, 512], FP32, space="PSUM",
                                          name="ps", tag="ps")
                            ps_pair.append(ps)
                        scatter_tile(g, i, ps_pair)
                        if (i + 1) % T == 0:
                            epilogue_chunk(g, i // T, ps_pair[i // T])
                    del state[g]
    split_sync_waits(nc, maxw=1)
    nc.finalize()
    return nc


def make_in_maps(p: Params, meta, x, w_proj, rs, ts, rds, w_out):
    H, F = p.H, p.F
    # Wcat' = 0.25 * W_h @ w_out  (out-proj + head-mean folded)
    wv = np.zeros((F + 1, 4 * F + 2 * H), np.float32)
    for h in range(H):
        wv[0:F, h * F:(h + 1) * F] = 0.25 * (w_proj[h] @ w_out)
        wv[0:F, 4 * F + h] = w_proj[h] @ ts[h]          # tangential V
        wv[0:F, 4 * F + H + h] = w_proj[h] @ rs[h]      # radial V
    wv[F, 4 * F + H:4 * F + 2 * H] = -float(rds)        # len bias on radial
    wm2 = (-0.5 * w_proj.sum(axis=0) @ w_out).astype(np.float32)
    in_maps = []
    for k in range(p.NC):
        c = meta['per_core'][k]
        in_maps.append({
            "xeT": c['xeT'], "ohT": c['ohT'],
            "wv": wv.astype(bf16), "wm2": wm2.astype(bf16),
            "x_rcvT": c['x_rcvT'], "x_rcv": c['x_rcv'],
        })
    return in_maps


def assemble(p: Params, meta, results):
    y = np.zeros((p.N, p.F), np.float32)
    for k in range(p.NC):
        c = meta['per_core'][k]
        y[k * p.RPC:(k + 1) * p.RPC] = results[k]["y_perm"][c['slot_global']]
    return y


install()

_CACHE = {}


def kernel(x, edge_index, edge_vec, edge_len, w_proj, radial_score,
           tangential_score, radial_distance_scale, w_out):
    x = np.asarray(x, np.float32)
    edge_index = np.asarray(edge_index)
    edge_len = np.asarray(edge_len, np.float32)
    w_proj = np.asarray(w_proj, np.float32)
    rs = np.asarray(radial_score, np.float32)
    ts = np.asarray(tangential_score, np.float32)
    rds = np.float32(np.asarray(radial_distance_scale))
    w_out_ = np.asarray(w_out, np.float32)

    N, F = x.shape
    H = w_proj.shape[0]
    E = edge_index.shape[1]
    p = Params(N, E, F=F, H=H, NC=8, GC=2)
    meta = host_prep(p, x, edge_index, edge_len)
    T = meta['T']
    key = (N, E, F, H, T)
    if key not in _CACHE:
        _CACHE[key] = build_program(p, T)
    nc = _CACHE[key]
    in_maps = make_in_maps(p, meta, x, w_proj, rs, ts, rds, w_out_)
    from concourse.bass_utils import run_bass_kernel_spmd
    res = run_bass_kernel_spmd(nc, in_maps, list(range(p.NC)))
    y = assemble(p, meta, [res.results[i] for i in range(p.NC)])
    return y.astype(np.float32)

